# revision 3
# baseline (speedup 1.0000x reference)
"""Cost-volume kernel for Trainium2 (8 NeuronCores, SPMD).

cost[b,c,h,x,d] = left[b,c,h,x] - right[b,c,h,x-d]  (0 where x < d)
with B,C,H,W = 4,32,128,240 and D = 24.

Sharding: every (b,c,h) row is independent, so flatten to 16384 rows of
W=240 and give each of the 8 cores a contiguous 2048-row block (pure
data parallelism, no halo).

Per 128-row SBUF tile the [W, D] cost slab is produced by 6 vector ops.
The output free-axis layout j = 24*w + d is fixed by HBM, and fp32
writes with stride 96 B run ~6x slower on the DVE than contiguous ones,
while strides <= 32 B run at full speed.  So d is split into 3 groups
of 8 (32 B-aligned, 8-contiguous runs):

  rect op (per group g):  w in [8g+7, 240), dg in [0,8):
      ot[24w + 8g + dg] = lt[w] - rt[w - 8g - dg]     (all valid)
  corner op (per group):  the remaining 56 valid cells w in [8g, 8g+7)
      via a sheared AP (dg, k'=w-8g-dg), strided but tiny.

Invalid positions (x < d, all with j < 552) are never written by either
op, so they are zeroed once per buffer at kernel start and persist.
The store DMA moves a fully contiguous [128, 5760] slab per tile.
"""

import sys

if "/opt/trn_rl_repo" not in sys.path:
    sys.path.insert(0, "/opt/trn_rl_repo")

import numpy as np

B, C, H, W, D = 4, 32, 128, 240, 24
P = 128
N_CORES = 8
ROWS = B * C * H                 # 16384
ROWS_PER_CORE = ROWS // N_CORES  # 2048
OTW = W * D                      # 5760

_nc_cache = None


def _build(K=1, NB=6, out_engines=("sync", "scalar"), in_engine="gpsimd"):
    from concourse import mybir, bacc
    import concourse.tile as tile
    import bass_rust

    f32 = mybir.dt.float32
    nc = bacc.Bacc("TRN2", target_bir_lowering=False, debug=False)
    left = nc.dram_tensor("left", [ROWS_PER_CORE, W], f32, kind="ExternalInput").ap()
    right = nc.dram_tensor("right", [ROWS_PER_CORE, W], f32, kind="ExternalInput").ap()
    out = nc.dram_tensor("out", [ROWS_PER_CORE, OTW], f32, kind="ExternalOutput").ap()
    ntiles = ROWS_PER_CORE // P  # 16
    ine = getattr(nc, in_engine)
    with tile.TileContext(nc) as tc:
        with tc.tile_pool(name="p", bufs=1) as pool:
            lts = [pool.tile([P, W], f32, name=f"lt{i}") for i in range(NB)]
            rts = [pool.tile([P, W], f32, name=f"rt{i}") for i in range(NB)]
            ots = [pool.tile([P, OTW], f32, name=f"ot{i}") for i in range(NB)]
            for i in range(NB):
                # invalid (x < d) positions all lie in [0, 552); zeroed once,
                # never overwritten by the valid-only compute ops below
                nc.vector.memset(ots[i][:, :552], 0.0)
            for k in range(K):
                for t in range(ntiles):
                    lt, rt, ot = lts[t % NB], rts[t % NB], ots[t % NB]
                    ine.dma_start(out=lt[:], in_=left[t * P:(t + 1) * P, :])
                    ine.dma_start(out=rt[:], in_=right[t * P:(t + 1) * P, :])
                    for g in range(3):
                        cw = W - (8 * g + 7)
                        o_ap = bass_rust.AP(tensor=ot[:].tensor, offset=200 * g + 168,
                                            ap=[[OTW, P], [24, cw], [1, 8]])
                        l_ap = bass_rust.AP(tensor=lt[:].tensor, offset=8 * g + 7,
                                            ap=[[W, P], [1, cw], [0, 8]])
                        r_ap = bass_rust.AP(tensor=rt[:].tensor, offset=7,
                                            ap=[[W, P], [1, cw], [-1, 8]])
                        nc.vector.tensor_sub(out=o_ap, in0=l_ap, in1=r_ap)
                        o2 = bass_rust.AP(tensor=ot[:].tensor, offset=200 * g,
                                          ap=[[OTW, P], [25, 8], [24, 7]])
                        l2 = bass_rust.AP(tensor=lt[:].tensor, offset=8 * g,
                                          ap=[[W, P], [1, 8], [1, 7]])
                        r2 = bass_rust.AP(tensor=rt[:].tensor, offset=0,
                                          ap=[[W, P], [0, 8], [1, 7]])
                        nc.vector.tensor_sub(out=o2, in0=l2, in1=r2)
                    oute = getattr(nc, out_engines[t % len(out_engines)])
                    oute.dma_start(out=out[t * P:(t + 1) * P, :], in_=ot[:])
    nc.compile()
    return nc


BUILD = _build


def _get_nc():
    global _nc_cache
    if _nc_cache is None:
        _nc_cache = _build()
    return _nc_cache


def kernel(left_img: np.ndarray, right_img: np.ndarray) -> np.ndarray:
    from concourse.bass_utils import run_bass_kernel_spmd

    nc = _get_nc()
    lf = np.ascontiguousarray(left_img, dtype=np.float32).reshape(ROWS, W)
    rf = np.ascontiguousarray(right_img, dtype=np.float32).reshape(ROWS, W)
    in_maps = []
    for i in range(N_CORES):
        sl = slice(i * ROWS_PER_CORE, (i + 1) * ROWS_PER_CORE)
        in_maps.append({"left": np.ascontiguousarray(lf[sl]),
                        "right": np.ascontiguousarray(rf[sl])})
    res = run_bass_kernel_spmd(nc, in_maps, list(range(N_CORES)))
    shards = [res.results[i]["out"] for i in range(N_CORES)]
    full = np.concatenate(shards, axis=0)
    return full.reshape(B, C, H, W, D)



# revision 7
# speedup vs baseline: 5.4582x; 5.4582x over previous
"""Cost-volume kernel for Trainium2 (8 NeuronCores, SPMD).

cost[b,c,h,x,d] = left[b,c,h,x] - right[b,c,h,x-d]  (0 where x < d)
with B,C,H,W = 4,32,128,240 and D = 24.

Sharding: every (b,c,h) row is independent -> flatten to 16384 rows of
W=240, each core gets a contiguous 2048-row block (pure data parallel).
Within a core, SBUF partition p of tile t holds DRAM row p*16 + t, so
every input load's DRAM side is a contiguous 3.8KB run per partition
(a p-major map would gather 480B strided runs, below the 512B DMA
line-rate knee); stores stay 11.5KB contiguous runs per row either way.

Device kernel (bf16, d-major):
  The output is produced in SBUF as ot[p, 240*d + w] (d-major), because
  that layout lets every DVE operand keep innermost stride +1 / 4B
  alignment, engaging the 2x_1P DVE mode (fp32 tensor_tensor would run
  1x and bf16 with w-major strided writes also falls back to 1x).
  Tolerance is 2e-2 so bf16 inputs/outputs are safe (~4e-3 rel err) and
  halve both DVE time and HBM store traffic.

  One shifted copy pair of `right` is packed in a single SBUF buffer:
    R2[p, t*268 + k]            = r[k-26]   (zeros for k<26)
    R2[p, 16*268 + t*268 + k]   = r[k-27]   (zeros for k<27)
  so cost[d=2i+q, w] = l[w] - R2[q*16*268 + t*268 + 26 + w - 2i] is ONE
  4D-AP tensor_sub per 128-row tile (even/odd d differ only by the
  constant stride 16*268; all APs stay 4B-aligned for every q,i).
  Invalid cells (w < d) read zeros from the pad and produce l[w]; the
  host zeroes them exactly afterwards.

  Stores are bitcast to fp32 (half the element count): plain bf16-typed
  HBM stores measured ~10x slower on HW, while the same bytes typed as
  fp32 run at full rate.

Host glue: round inputs to bf16 (RNE via integer ops), run via
run_bass_kernel_spmd, zero invalid cells, upcast bf16->f32 with integer
shifts, and return the (B,C,H,W,D) transposed view (d-major -> w-major
is a zero-copy stride permutation).
"""

import sys

if "/opt/trn_rl_repo" not in sys.path:
    sys.path.insert(0, "/opt/trn_rl_repo")

import numpy as np

B, C, H, W, D = 4, 32, 128, 240, 24
P = 128
N_CORES = 8
ROWS = B * C * H                 # 16384
ROWS_PER_CORE = ROWS // N_CORES  # 2048
NT = ROWS_PER_CORE // P          # 16 tiles per core
OTW = W * D                      # 5760
RST = 268                        # padded per-tile row stride in R2

_nc_cache = None


def _build(K=1, NB=6, out_engines=("sync", "scalar"),
           in_engines=("gpsimd",), loop=0, halves=2):
    from concourse import mybir, bacc
    import concourse.tile as tile
    import bass_rust

    if isinstance(in_engines, str):
        in_engines = (in_engines,)
    if isinstance(out_engines, str):
        out_engines = (out_engines,)
    bf = mybir.dt.bfloat16
    f32 = mybir.dt.float32
    nc = bacc.Bacc("TRN2", target_bir_lowering=False, debug=False)
    left = nc.dram_tensor("left", [ROWS_PER_CORE, W], bf,
                          kind="ExternalInput").ap()
    right = nc.dram_tensor("right", [ROWS_PER_CORE, W], bf,
                           kind="ExternalInput").ap()
    out = nc.dram_tensor("out", [ROWS_PER_CORE, OTW // 2], f32,
                         kind="ExternalOutput").ap()
    LS = NT * W
    HNT = NT * RST
    RS2 = 2 * HNT
    with tile.TileContext(nc) as tc:
        with tc.tile_pool(name="p", bufs=1) as pool:
            Lb = pool.tile([P, LS], bf, name="Lb")
            R2 = pool.tile([P, RS2], bf, name="R2")
            # zero only the pad cells the shifted copies read (k<26 / k<27)
            nc.vector.memset(bass_rust.AP(tensor=R2[:].tensor, offset=0,
                                          ap=[[RS2, P], [RST, NT], [1, 26]]),
                             0.0)
            nc.vector.memset(bass_rust.AP(tensor=R2[:].tensor, offset=HNT,
                                          ap=[[RS2, P], [RST, NT], [1, 27]]),
                             0.0)
            ots = [pool.tile([P, OTW], bf, name=f"ot{i}") for i in range(NB)]

            OW2 = OTW // 2

            def body():
                tph = NT // halves
                li = 0
                for h in range(halves):
                    ine = getattr(nc, in_engines[li % len(in_engines)])
                    li += 1
                    ine.dma_start(
                        out=bass_rust.AP(tensor=Lb[:].tensor,
                                         offset=h * tph * W,
                                         ap=[[LS, P], [W, tph], [1, W]]),
                        in_=bass_rust.AP(tensor=left.tensor,
                                         offset=h * tph * W,
                                         ap=[[LS, P], [W, tph], [1, W]]))
                    for roff in (26, HNT + 27):
                        ine = getattr(nc, in_engines[li % len(in_engines)])
                        li += 1
                        ine.dma_start(
                            out=bass_rust.AP(tensor=R2[:].tensor,
                                             offset=h * tph * RST + roff,
                                             ap=[[RS2, P], [RST, tph], [1, W]]),
                            in_=bass_rust.AP(tensor=right.tensor,
                                             offset=h * tph * W,
                                             ap=[[LS, P], [W, tph], [1, W]]))
                for t in range(NT):
                    ot = ots[t % NB]
                    nc.vector.tensor_sub(
                        out=bass_rust.AP(tensor=ot[:].tensor, offset=0,
                                         ap=[[OTW, P], [240, 2], [480, 12],
                                             [1, 240]]),
                        in0=bass_rust.AP(tensor=Lb[:].tensor, offset=t * W,
                                         ap=[[LS, P], [0, 2], [0, 12],
                                             [1, 240]]),
                        in1=bass_rust.AP(tensor=R2[:].tensor,
                                         offset=t * RST + 26,
                                         ap=[[RS2, P], [HNT, 2], [-2, 12],
                                             [1, 240]]))
                    oute = getattr(nc, out_engines[t % len(out_engines)])
                    oute.dma_start(
                        out=bass_rust.AP(tensor=out.tensor, offset=t * OW2,
                                         ap=[[NT * OW2, P], [1, OW2]]),
                        in_=ot[:].bitcast(f32))

            if loop:
                with tc.For_i(0, K, 1):
                    body()
            else:
                for _ in range(K):
                    body()
    nc.compile()
    return nc


BUILD = _build


def _get_nc():
    global _nc_cache
    if _nc_cache is None:
        _nc_cache = _build()
    return _nc_cache


def _f32_to_bf16_rne(x: np.ndarray) -> np.ndarray:
    """Round-to-nearest-even f32 -> bf16, returned as a uint16 array."""
    u = np.ascontiguousarray(x, dtype=np.float32).view(np.uint32)
    return ((u + 0x7FFF + ((u >> 16) & 1)) >> 16).astype(np.uint16)


def kernel(left_img: np.ndarray, right_img: np.ndarray) -> np.ndarray:
    from concourse.bass_utils import run_bass_kernel_spmd
    import ml_dtypes

    nc = _get_nc()
    lb = _f32_to_bf16_rne(left_img).reshape(ROWS, W).view(ml_dtypes.bfloat16)
    rb = _f32_to_bf16_rne(right_img).reshape(ROWS, W).view(ml_dtypes.bfloat16)
    in_maps = []
    for i in range(N_CORES):
        sl = slice(i * ROWS_PER_CORE, (i + 1) * ROWS_PER_CORE)
        in_maps.append({"left": np.ascontiguousarray(lb[sl]),
                        "right": np.ascontiguousarray(rb[sl])})
    res = run_bass_kernel_spmd(nc, in_maps, list(range(N_CORES)))
    shards = [np.asarray(res.results[i]["out"]) for i in range(N_CORES)]
    full = np.concatenate(shards, axis=0)          # [ROWS, 2880] f32 container
    ob = np.ascontiguousarray(full).view(np.uint16).reshape(ROWS, D, W)
    for d in range(1, D):                          # exact zeros where x < d
        ob[:, d, :d] = 0
    u32 = ob.astype(np.uint32)
    u32 <<= 16
    f = u32.view(np.float32)                       # [ROWS, D, W] fp32
    return f.reshape(B, C, H, D, W).transpose(0, 1, 2, 4, 3)
